# revision 22
# baseline (speedup 1.0000x reference)
"""MoE (top-2 of 8 experts, GELU MLP) on 8 Trainium2 NeuronCores.

Sharding: expert-parallel, one expert per core (hint: "shard W1/W2 along the
expert axis across M devices with all-to-all token dispatch/combine").
The host plays the role of the all-to-all fabric: it routes tokens (softmax
top-2 over the gate logits, computed in float64 -- the reference's selection
margins are >>fp32 noise so selection is exact), gathers each expert's tokens
to a padded capacity, and after the device pass combines the two expert
outputs per token with the routing weights. All heavy compute (>99.9% of
FLOPs: both 4096-wide GEMMs + exact erf-GELU) runs on the NeuronCores in
fp32r (full PE rate, ~1e-4 rms error vs fp32).

Device kernel per core e (SPMD, same program, different data):
    y = gelu(x_e @ W1[e]) @ W2[e]
with x_e fed transposed ([C, cap]) so GEMM1 produces h^T directly
(stationary = W1 tiles) and GEMM2 (stationary = h^T tiles, moving = W2
panels) produces token-major y without any on-device transposes.
"""

import sys

if "/opt/trn_rl_repo" not in sys.path:
    sys.path.insert(0, "/opt/trn_rl_repo")

import numpy as np

import concourse.bass as bass  # noqa: F401  (registers engine types)
import concourse.mybir as mybir
import concourse.tile as tile
from concourse import bacc
from concourse.bass_utils import run_bass_kernel_spmd

N_CORES = 8
C = 1024          # n_embd
E = 8             # n_experts
F = 4096          # d_ff
TOP_K = 2
KC = C // 128     # 8 k-tiles for GEMM1
KF = F // 128     # 32 k-tiles for GEMM2
F32 = mybir.dt.float32
F32R = mybir.dt.float32r

LAST_EXEC_TIME_NS = None      # set when tracing is enabled (see test harness)
LAST_RESULTS = None


def _install_axon_ntff_shim():
    """This image's `antenv` lacks `axon_hooks`; polyfill it so
    run_bass_kernel_spmd(trace=True) (or env BASS_TRACE=1) works instead of
    crashing on import. Registers the real ctypes NTFF hook when available."""
    import types

    try:
        import antenv
    except ImportError:
        return
    if hasattr(antenv, "axon_hooks"):
        return
    mod = types.ModuleType("antenv.axon_hooks")
    mod._hook = None

    def set_axon_ntff_profile_hook(h):
        mod._hook = h

    def get_axon_ntff_profile_hook():
        return mod._hook

    mod.set_axon_ntff_profile_hook = set_axon_ntff_profile_hook
    mod.get_axon_ntff_profile_hook = get_axon_ntff_profile_hook
    sys.modules["antenv.axon_hooks"] = mod
    antenv.axon_hooks = mod
    try:
        from trn_agent_boot.trn_boot import _ntff_profile_via_ctypes

        mod.set_axon_ntff_profile_hook(
            _ntff_profile_via_ctypes("/opt/axon/libaxon_pjrt.so")
        )
    except Exception:
        pass


_install_axon_ntff_shim()


def _chunks_for(maxcnt: int) -> list[int]:
    """Token chunks: each in {256, 384, 512} (PSUM bank = 512 fp32; fp32r
    needs moving dim >= 256 for full PE rate), multiples of 128, summing to
    >= maxcnt with minimal padding."""
    cap = max(256, -(-maxcnt // 128) * 128)
    windows = []
    rem = cap
    while rem > 768:
        if rem <= 1280:              # near-equal split keeps the per-window
            hi = -(-rem // 256) * 128    # W-stream DMA demand smooth
            windows.extend([hi, rem - hi])
            rem = 0
            break
        windows.append(768)
        rem -= 768
    if rem:
        windows.append(rem)
    assert sum(windows) == cap
    assert all(w % 128 == 0 and 256 <= w <= 768 for w in windows), windows
    return windows


def _subs_for(wn: int) -> list[int]:
    """GEMM1 PSUM sub-chunks: <=512 (one fp32 bank), >=256 (fp32r full rate)."""
    subs = []
    rem = wn
    while rem >= 512 + 256:
        subs.append(512)
        rem -= 512
    if rem > 512:
        subs.extend([rem - 256, 256])
    else:
        subs.append(rem)
    assert sum(subs) == wn and all(256 <= s <= 512 for s in subs)
    # Ascending: the first PSUM group then depends on the smallest x slice,
    # which shortens the kernel-start DMA fill before the first matmul.
    return sorted(subs)


def _build(chunks: list[int]):
    """Build the per-core Bass program: y[cap, C] = gelu(xT.T @ W1) @ W2."""
    cap = sum(chunks)
    nc = bacc.Bacc("TRN2", target_bir_lowering=False, debug=False)
    xT = nc.dram_tensor("xT", [C, cap], F32R, kind="ExternalInput")
    w1 = nc.dram_tensor("w1", [C, F], F32R, kind="ExternalInput")
    w2 = nc.dram_tensor("w2", [F, C], F32R, kind="ExternalInput")
    y = nc.dram_tensor("y", [cap, C], F32, kind="ExternalOutput")
    gelu = mybir.ActivationFunctionType.Gelu

    with tile.TileContext(nc) as tc:
        with (
            tc.tile_pool(name="xp", bufs=1) as xp,
            tc.tile_pool(name="w1p", bufs=3) as w1p,
            tc.tile_pool(name="w2p", bufs=4) as w2p,
            tc.tile_pool(name="hp", bufs=1) as hp,
            tc.tile_pool(name="yp", bufs=1) as yp,
            tc.tile_pool(name="ps1", bufs=3, space="PSUM") as ps1,
            tc.tile_pool(name="ps2", bufs=3, space="PSUM") as ps2,
        ):
            t0 = 0
            first_window = True
            for wn in chunks:
                nm = wn // 128
                subs = _subs_for(wn)
                # ---- load this window's tokens: [C, wn] -> [128, KC, wn]
                x_sb = xp.tile([128, KC * wn], F32R, tag="x")
                s0 = 0
                for sn in _subs_for(wn):   # sub-granular so the first GEMM1
                    nc.sync.dma_start(     # PSUM group can start early
                        out=x_sb[:].rearrange("p (k n) -> p k n", k=KC)[:, :, s0:s0 + sn],
                        in_=xT.ap()[:, t0 + s0:t0 + s0 + sn].rearrange(
                            "(k p) n -> p k n", p=128
                        ),
                    )
                    s0 += sn

                # ---- GEMM1 + GELU: hT[f, tokens] = gelu(W1.T @ x)
                # W1 streamed once per window in 256-wide panels.
                h_tiles = []
                for fo in range(F // 256):
                    w1_sb = w1p.tile([128, KC * 256], F32R, tag="w1")
                    # First panels of the first window ride the scalar-engine
                    # HWDGE ring, parallel to the sync ring carrying x, to
                    # shorten the kernel-start fill before the first matmul.
                    dma_eng = nc.scalar if (first_window and fo < 2) else nc.sync
                    dma_eng.dma_start(
                        out=w1_sb[:].rearrange("p (k f) -> p k f", k=KC),
                        in_=w1.ap()[:, fo * 256:(fo + 1) * 256].rearrange(
                            "(k p) f -> p k f", p=128
                        ),
                    )
                    for fi in range(2):
                        hT = hp.tile([128, wn], F32R, tag=f"h{fo * 2 + fi}")
                        s0 = 0
                        for sn in subs:
                            ph = ps1.tile([128, sn], F32, tag="ph")
                            for k in range(KC):
                                nc.tensor.matmul(
                                    ph[:],
                                    lhsT=w1_sb[:, k * 256 + fi * 128:
                                               k * 256 + (fi + 1) * 128],
                                    rhs=x_sb[:, k * wn + s0:k * wn + s0 + sn],
                                    start=(k == 0),
                                    stop=(k == KC - 1),
                                )
                            nc.scalar.activation(hT[:, s0:s0 + sn], ph[:], gelu)
                            s0 += sn
                        h_tiles.append(hT)

                # ---- GEMM2: y[tok, co*512:+512] = hT.T @ W2 half, W2
                # streamed once per window in k-groups of 4; PSUM rotates per
                # (kg, m) group and partials accumulate in SBUF via DVE.
                for co in range(2):
                    yacc = [
                        yp.tile([128, 512], F32, tag=f"yam{m}", name=f"yam{m}")
                        for m in range(nm)
                    ]
                    for kg in range(KF // 4):
                        w2g = w2p.tile([128, 4 * 512], F32R, tag="w2g")
                        nc.sync.dma_start(
                            out=w2g[:].rearrange("p (k c) -> p k c", k=4),
                            in_=w2.ap()[kg * 512:(kg + 1) * 512,
                                        co * 512:(co + 1) * 512].rearrange(
                                "(k p) c -> p k c", p=128
                            ),
                        )
                        for m in range(nm):
                            py = ps2.tile([128, 512], F32, tag="py")
                            for j in range(4):
                                k = kg * 4 + j
                                nc.tensor.matmul(
                                    py[:],
                                    lhsT=h_tiles[k][:, m * 128:(m + 1) * 128],
                                    rhs=w2g[:, j * 512:(j + 1) * 512],
                                    start=(j == 0),
                                    stop=(j == 3),
                                )
                            if kg == 0:
                                nc.vector.tensor_copy(yacc[m][:], py[:])
                            else:
                                nc.vector.tensor_add(yacc[m][:], yacc[m][:], py[:])
                    for m in range(nm):
                        nc.sync.dma_start(
                            out=y.ap()[t0 + m * 128:t0 + (m + 1) * 128,
                                       co * 512:(co + 1) * 512],
                            in_=yacc[m][:],
                        )
                t0 += wn
                first_window = False
    nc.compile()
    return nc


def kernel(x, Wg, W1, W2):
    global LAST_EXEC_TIME_NS, LAST_RESULTS
    x = np.asarray(x, dtype=np.float32)
    Wg = np.asarray(Wg, dtype=np.float32)
    W1 = np.asarray(W1, dtype=np.float32)
    W2 = np.asarray(W2, dtype=np.float32)
    B, T, _ = x.shape
    ntok = B * T
    xf = x.reshape(ntok, C)

    # ---- router (replicated gate, fp64 for stable selection)
    logits = xf.astype(np.float64) @ Wg.astype(np.float64)
    logits -= logits.max(-1, keepdims=True)
    probs = np.exp(logits)
    probs /= probs.sum(-1, keepdims=True)
    top2 = np.argsort(-probs, axis=-1, kind="stable")[:, :TOP_K]       # [ntok, 2]
    w12 = np.take_along_axis(probs, top2, axis=-1)
    w12 = w12 / w12.sum(-1, keepdims=True)                             # [ntok, 2]

    # aux load-balancing loss
    f_frac = np.bincount(top2.ravel(), minlength=E) / (ntok * TOP_K)
    P_mean = probs.mean(axis=0)
    aux_loss = np.float32(E * (f_frac * P_mean).sum())

    # ---- dispatch: gather each expert's tokens, pad to shared capacity
    token_lists = [np.nonzero((top2 == e).any(-1))[0] for e in range(E)]
    maxcnt = max(len(t) for t in token_lists)
    chunks = _chunks_for(maxcnt)
    cap = sum(chunks)

    in_maps = []
    for e in range(E):
        tl = token_lists[e]
        xe = np.zeros((C, cap), np.float32)
        xe[:, :len(tl)] = xf[tl].T
        in_maps.append({
            "xT": xe,
            "w1": np.ascontiguousarray(W1[e]),
            "w2": np.ascontiguousarray(W2[e]),
        })

    nc = _build(chunks)
    res = run_bass_kernel_spmd(nc, in_maps, list(range(N_CORES)))
    LAST_EXEC_TIME_NS = res.exec_time_ns
    LAST_RESULTS = res

    # ---- combine: out[t] = sum_k w12[t,k] * y_{expert k}[t]
    out = np.zeros((ntok, C), np.float64)
    for e in range(E):
        tl = token_lists[e]
        ye = res.results[e]["y"][:len(tl)].astype(np.float64)
        we = np.where(top2[tl, 0] == e, w12[tl, 0], w12[tl, 1])[:, None]
        out[tl] += we * ye
    return out.reshape(B, T, C).astype(np.float32), aux_loss


# revision 26
# speedup vs baseline: 1.0121x; 1.0121x over previous
"""MoE (top-2 of 8 experts, GELU MLP) on 8 Trainium2 NeuronCores.

Sharding: expert-parallel, one expert per core (hint: "shard W1/W2 along the
expert axis across M devices with all-to-all token dispatch/combine").
The host plays the role of the all-to-all fabric: it routes tokens (softmax
top-2 over the gate logits, computed in float64 -- the reference's selection
margins are >>fp32 noise so selection is exact), gathers each expert's tokens
to a padded capacity, and after the device pass combines the two expert
outputs per token with the routing weights. All heavy compute (>99.9% of
FLOPs: both 4096-wide GEMMs + exact erf-GELU) runs on the NeuronCores in
fp32r (full PE rate, ~1e-4 rms error vs fp32).

Device kernel per core e (SPMD, same program, different data):
    y = gelu(x_e @ W1[e]) @ W2[e]
with x_e fed transposed ([C, cap]) so GEMM1 produces h^T directly
(stationary = W1 tiles) and GEMM2 (stationary = h^T tiles, moving = W2
panels) produces token-major y without any on-device transposes.
"""

import sys

if "/opt/trn_rl_repo" not in sys.path:
    sys.path.insert(0, "/opt/trn_rl_repo")

import numpy as np

import concourse.bass as bass  # noqa: F401  (registers engine types)
import concourse.mybir as mybir
import concourse.tile as tile
from concourse import bacc
from concourse.bass_utils import run_bass_kernel_spmd

N_CORES = 8
C = 1024          # n_embd
E = 8             # n_experts
F = 4096          # d_ff
TOP_K = 2
KC = C // 128     # 8 k-tiles for GEMM1
KF = F // 128     # 32 k-tiles for GEMM2
F32 = mybir.dt.float32
F32R = mybir.dt.float32r

LAST_EXEC_TIME_NS = None      # set when tracing is enabled (see test harness)
LAST_RESULTS = None


def _install_axon_ntff_shim():
    """This image's `antenv` lacks `axon_hooks`; polyfill it so
    run_bass_kernel_spmd(trace=True) (or env BASS_TRACE=1) works instead of
    crashing on import. Registers the real ctypes NTFF hook when available."""
    import types

    try:
        import antenv
    except ImportError:
        return
    if hasattr(antenv, "axon_hooks"):
        return
    mod = types.ModuleType("antenv.axon_hooks")
    mod._hook = None

    def set_axon_ntff_profile_hook(h):
        mod._hook = h

    def get_axon_ntff_profile_hook():
        return mod._hook

    mod.set_axon_ntff_profile_hook = set_axon_ntff_profile_hook
    mod.get_axon_ntff_profile_hook = get_axon_ntff_profile_hook
    sys.modules["antenv.axon_hooks"] = mod
    antenv.axon_hooks = mod
    try:
        from trn_agent_boot.trn_boot import _ntff_profile_via_ctypes

        mod.set_axon_ntff_profile_hook(
            _ntff_profile_via_ctypes("/opt/axon/libaxon_pjrt.so")
        )
    except Exception:
        pass


_install_axon_ntff_shim()


def _chunks_for(maxcnt: int) -> list[int]:
    """Token chunks: each in {256, 384, 512} (PSUM bank = 512 fp32; fp32r
    needs moving dim >= 256 for full PE rate), multiples of 128, summing to
    >= maxcnt with minimal padding."""
    cap = max(256, -(-maxcnt // 128) * 128)
    windows = []
    rem = cap
    while rem > 768:
        if rem <= 1280:              # near-equal split keeps the per-window
            hi = -(-rem // 256) * 128    # W-stream DMA demand smooth
            windows.extend([hi, rem - hi])
            rem = 0
            break
        windows.append(768)
        rem -= 768
    if rem:
        windows.append(rem)
    assert sum(windows) == cap
    assert all(w % 128 == 0 and 256 <= w <= 768 for w in windows), windows
    return windows


def _subs_for(wn: int) -> list[int]:
    """GEMM1 PSUM sub-chunks: <=512 (one fp32 bank), >=256 (fp32r full rate)."""
    subs = []
    rem = wn
    while rem >= 512 + 256:
        subs.append(512)
        rem -= 512
    if rem > 512:
        subs.extend([rem - 256, 256])
    else:
        subs.append(rem)
    assert sum(subs) == wn and all(256 <= s <= 512 for s in subs)
    # Ascending: the first PSUM group then depends on the smallest x slice,
    # which shortens the kernel-start DMA fill before the first matmul.
    return sorted(subs)


def _build(chunks: list[int]):
    """Build the per-core Bass program: y[cap, C] = gelu(xT.T @ W1) @ W2."""
    cap = sum(chunks)
    nc = bacc.Bacc("TRN2", target_bir_lowering=False, debug=False)
    xT = nc.dram_tensor("xT", [C, cap], F32R, kind="ExternalInput")
    w1 = nc.dram_tensor("w1", [C, F], F32R, kind="ExternalInput")
    w2 = nc.dram_tensor("w2", [F, C], F32R, kind="ExternalInput")
    y = nc.dram_tensor("y", [cap, C], F32, kind="ExternalOutput")
    gelu = mybir.ActivationFunctionType.Gelu

    with tile.TileContext(nc) as tc:
        with (
            tc.tile_pool(name="xp", bufs=1) as xp,
            tc.tile_pool(name="w1p", bufs=3) as w1p,
            tc.tile_pool(name="w2p", bufs=4) as w2p,
            tc.tile_pool(name="hp", bufs=1) as hp,
            tc.tile_pool(name="yp", bufs=1) as yp,
            tc.tile_pool(name="ps1", bufs=3, space="PSUM") as ps1,
            tc.tile_pool(name="ps2", bufs=3, space="PSUM") as ps2,
        ):
            t0 = 0
            first_window = True
            for wn in chunks:
                nm = wn // 128
                subs = _subs_for(wn)
                # ---- load this window's tokens: [C, wn] -> [128, KC, wn]
                x_sb = xp.tile([128, KC * wn], F32R, tag="x")
                s0 = 0
                for sn in _subs_for(wn):   # sub-granular so the first GEMM1
                    nc.sync.dma_start(     # PSUM group can start early
                        out=x_sb[:].rearrange("p (k n) -> p k n", k=KC)[:, :, s0:s0 + sn],
                        in_=xT.ap()[:, t0 + s0:t0 + s0 + sn].rearrange(
                            "(k p) n -> p k n", p=128
                        ),
                    )
                    s0 += sn

                # ---- GEMM1 + GELU: hT[f, tokens] = gelu(W1.T @ x)
                # W1 streamed once per window in 256-wide panels.
                h_tiles = []
                for fo in range(F // 256):
                    w1_sb = w1p.tile([128, KC * 256], F32R, tag="w1")
                    # First panels of the first window ride the scalar-engine
                    # HWDGE ring, parallel to the sync ring carrying x, to
                    # shorten the kernel-start fill before the first matmul.
                    dma_eng = nc.scalar if (first_window and fo < 2) else nc.sync
                    dma_eng.dma_start(
                        out=w1_sb[:].rearrange("p (k f) -> p k f", k=KC),
                        in_=w1.ap()[:, fo * 256:(fo + 1) * 256].rearrange(
                            "(k p) f -> p k f", p=128
                        ),
                    )
                    for fi in range(2):
                        hT = hp.tile([128, wn], F32R, tag=f"h{fo * 2 + fi}")
                        s0 = 0
                        for sn in subs:
                            ph = ps1.tile([128, sn], F32, tag="ph")
                            for k in range(KC):
                                nc.tensor.matmul(
                                    ph[:],
                                    lhsT=w1_sb[:, k * 256 + fi * 128:
                                               k * 256 + (fi + 1) * 128],
                                    rhs=x_sb[:, k * wn + s0:k * wn + s0 + sn],
                                    start=(k == 0),
                                    stop=(k == KC - 1),
                                )
                            nc.scalar.activation(hT[:, s0:s0 + sn], ph[:], gelu)
                            s0 += sn
                        h_tiles.append(hT)

                # ---- GEMM2: y[tok, co*512:+512] = hT.T @ W2 half, W2
                # streamed once per window in k-groups of 4; PSUM rotates per
                # (kg, m) group and partials accumulate in SBUF via DVE.
                for co in range(2):
                    yacc = [
                        yp.tile([128, 512], F32, tag=f"yam{m}", name=f"yam{m}")
                        for m in range(nm)
                    ]
                    for kg in range(KF // 4):
                        w2g = w2p.tile([128, 4 * 512], F32R, tag="w2g")
                        nc.sync.dma_start(
                            out=w2g[:].rearrange("p (k c) -> p k c", k=4),
                            in_=w2.ap()[kg * 512:(kg + 1) * 512,
                                        co * 512:(co + 1) * 512].rearrange(
                                "(k p) c -> p k c", p=128
                            ),
                        )
                        for m in range(nm):
                            py = ps2.tile([128, 512], F32, tag="py")
                            for j in range(4):
                                k = kg * 4 + j
                                nc.tensor.matmul(
                                    py[:],
                                    lhsT=h_tiles[k][:, m * 128:(m + 1) * 128],
                                    rhs=w2g[:, j * 512:(j + 1) * 512],
                                    start=(j == 0),
                                    stop=(j == 3),
                                )
                            if kg == 0:
                                nc.vector.tensor_copy(yacc[m][:], py[:])
                            else:
                                nc.vector.tensor_add(yacc[m][:], yacc[m][:], py[:])
                    for m in range(nm):
                        nc.sync.dma_start(
                            out=y.ap()[t0 + m * 128:t0 + (m + 1) * 128,
                                       co * 512:(co + 1) * 512],
                            in_=yacc[m][:],
                        )
                t0 += wn
                first_window = False
    nc.compile()
    return nc


def kernel(x, Wg, W1, W2):
    global LAST_EXEC_TIME_NS, LAST_RESULTS
    x = np.asarray(x, dtype=np.float32)
    Wg = np.asarray(Wg, dtype=np.float32)
    W1 = np.asarray(W1, dtype=np.float32)
    W2 = np.asarray(W2, dtype=np.float32)
    B, T, _ = x.shape
    ntok = B * T
    xf = x.reshape(ntok, C)

    # ---- router (replicated gate, fp64 for stable selection)
    logits = xf.astype(np.float64) @ Wg.astype(np.float64)
    logits -= logits.max(-1, keepdims=True)
    probs = np.exp(logits)
    probs /= probs.sum(-1, keepdims=True)
    top2 = np.argsort(-probs, axis=-1, kind="stable")[:, :TOP_K]       # [ntok, 2]
    w12 = np.take_along_axis(probs, top2, axis=-1)
    w12 = w12 / w12.sum(-1, keepdims=True)                             # [ntok, 2]

    # aux load-balancing loss
    f_frac = np.bincount(top2.ravel(), minlength=E) / (ntok * TOP_K)
    P_mean = probs.mean(axis=0)
    aux_loss = np.float32(E * (f_frac * P_mean).sum())

    # ---- dispatch: gather each expert's tokens, pad to shared capacity
    token_lists = [np.nonzero((top2 == e).any(-1))[0] for e in range(E)]
    maxcnt = max(len(t) for t in token_lists)
    chunks = _chunks_for(maxcnt)
    cap = sum(chunks)

    in_maps = []
    for e in range(E):
        tl = token_lists[e]
        xe = np.zeros((C, cap), np.float32)
        xe[:, :len(tl)] = xf[tl].T
        in_maps.append({
            "xT": xe,
            "w1": np.ascontiguousarray(W1[e]),
            "w2": np.ascontiguousarray(W2[e]),
        })

    nc = _build(chunks)
    res = run_bass_kernel_spmd(nc, in_maps, list(range(N_CORES)))
    LAST_EXEC_TIME_NS = res.exec_time_ns
    LAST_RESULTS = res

    # ---- combine: out[t] = sum_k w12[t,k] * y_{expert k}[t]
    out = np.zeros((ntok, C), np.float64)
    for e in range(E):
        tl = token_lists[e]
        ye = res.results[e]["y"][:len(tl)].astype(np.float64)
        we = np.where(top2[tl, 0] == e, w12[tl, 0], w12[tl, 1])[:, None]
        out[tl] += we * ye
    return out.reshape(B, T, C).astype(np.float32), aux_loss


# revision 28
# speedup vs baseline: 1.0348x; 1.0224x over previous
"""MoE (top-2 of 8 experts, GELU MLP) on 8 Trainium2 NeuronCores.

Sharding: expert-parallel, one expert per core (hint: "shard W1/W2 along the
expert axis across M devices with all-to-all token dispatch/combine").
The host plays the role of the all-to-all fabric: it routes tokens (softmax
top-2 over the gate logits, computed in float64 -- the reference's selection
margins are >>fp32 noise so selection is exact), gathers each expert's tokens
to a padded capacity, and after the device pass combines the two expert
outputs per token with the routing weights. All heavy compute (>99.9% of
FLOPs: both 4096-wide GEMMs + exact erf-GELU) runs on the NeuronCores in
fp32r (full PE rate, ~1e-4 rms error vs fp32).

Device kernel per core e (SPMD, same program, different data):
    y = gelu(x_e @ W1[e]) @ W2[e]
with x_e fed transposed ([C, cap]) so GEMM1 produces h^T directly
(stationary = W1 tiles) and GEMM2 (stationary = h^T tiles, moving = W2
panels) produces token-major y without any on-device transposes.
"""

import sys

if "/opt/trn_rl_repo" not in sys.path:
    sys.path.insert(0, "/opt/trn_rl_repo")

import numpy as np

import concourse.bass as bass  # noqa: F401  (registers engine types)
import concourse.mybir as mybir
import concourse.tile as tile
from concourse import bacc
from concourse.bass_utils import run_bass_kernel_spmd

N_CORES = 8
C = 1024          # n_embd
E = 8             # n_experts
F = 4096          # d_ff
TOP_K = 2
KC = C // 128     # 8 k-tiles for GEMM1
KF = F // 128     # 32 k-tiles for GEMM2
F32 = mybir.dt.float32
F32R = mybir.dt.float32r

LAST_EXEC_TIME_NS = None      # set when tracing is enabled (see test harness)
LAST_RESULTS = None


def _install_axon_ntff_shim():
    """This image's `antenv` lacks `axon_hooks`; polyfill it so
    run_bass_kernel_spmd(trace=True) (or env BASS_TRACE=1) works instead of
    crashing on import. Registers the real ctypes NTFF hook when available."""
    import types

    try:
        import antenv
    except ImportError:
        return
    if hasattr(antenv, "axon_hooks"):
        return
    mod = types.ModuleType("antenv.axon_hooks")
    mod._hook = None

    def set_axon_ntff_profile_hook(h):
        mod._hook = h

    def get_axon_ntff_profile_hook():
        return mod._hook

    mod.set_axon_ntff_profile_hook = set_axon_ntff_profile_hook
    mod.get_axon_ntff_profile_hook = get_axon_ntff_profile_hook
    sys.modules["antenv.axon_hooks"] = mod
    antenv.axon_hooks = mod
    try:
        from trn_agent_boot.trn_boot import _ntff_profile_via_ctypes

        mod.set_axon_ntff_profile_hook(
            _ntff_profile_via_ctypes("/opt/axon/libaxon_pjrt.so")
        )
    except Exception:
        pass


_install_axon_ntff_shim()


def _chunks_for(maxcnt: int) -> list[int]:
    """Token windows: multiples of 128 in [256, 768] (h^T SBUF residency
    bounds a window at 768), summing to maxcnt rounded up to 128, split
    near-equally so the once-per-window W1/W2 streams keep DMA demand
    smooth across the kernel."""
    cap = max(256, -(-maxcnt // 128) * 128)
    windows = []
    rem = cap
    while rem > 768:
        if rem <= 1280:              # near-equal split keeps the per-window
            hi = -(-rem // 256) * 128    # W-stream DMA demand smooth
            windows.extend([hi, rem - hi])
            rem = 0
            break
        windows.append(768)
        rem -= 768
    if rem:
        windows.append(rem)
    assert sum(windows) == cap
    assert all(w % 128 == 0 and 256 <= w <= 768 for w in windows), windows
    return windows


def _subs_for(wn: int) -> list[int]:
    """GEMM1 PSUM sub-chunks: <=512 (one fp32 bank), >=256 (fp32r full rate)."""
    subs = []
    rem = wn
    while rem >= 512 + 256:
        subs.append(512)
        rem -= 512
    if rem > 512:
        subs.extend([rem - 256, 256])
    else:
        subs.append(rem)
    assert sum(subs) == wn and all(256 <= s <= 512 for s in subs)
    # Ascending: the first PSUM group then depends on the smallest x slice,
    # which shortens the kernel-start DMA fill before the first matmul.
    return sorted(subs)


def _build(chunks: list[int]):
    """Build the per-core Bass program: y[cap, C] = gelu(xT.T @ W1) @ W2."""
    cap = sum(chunks)
    nc = bacc.Bacc("TRN2", target_bir_lowering=False, debug=False)
    xT = nc.dram_tensor("xT", [C, cap], F32R, kind="ExternalInput")
    w1 = nc.dram_tensor("w1", [C, F], F32R, kind="ExternalInput")
    w2 = nc.dram_tensor("w2", [F, C], F32R, kind="ExternalInput")
    y = nc.dram_tensor("y", [cap, C], F32, kind="ExternalOutput")
    gelu = mybir.ActivationFunctionType.Gelu

    with tile.TileContext(nc) as tc:
        with (
            tc.tile_pool(name="xp", bufs=1) as xp,
            tc.tile_pool(name="w1p", bufs=3) as w1p,
            tc.tile_pool(name="w2p", bufs=4) as w2p,
            tc.tile_pool(name="hp", bufs=1) as hp,
            tc.tile_pool(name="yp", bufs=1) as yp,
            tc.tile_pool(name="ps1", bufs=3, space="PSUM") as ps1,
            tc.tile_pool(name="ps2", bufs=3, space="PSUM") as ps2,
        ):
            t0 = 0
            first_window = True
            for wn in chunks:
                nm = wn // 128
                subs = _subs_for(wn)
                # ---- load this window's tokens: [C, wn] -> [128, KC, wn]
                x_sb = xp.tile([128, KC * wn], F32R, tag="x")
                s0 = 0
                for sn in subs:            # sub-granular so the first GEMM1
                    nc.sync.dma_start(     # PSUM group can start early
                        out=x_sb[:].rearrange("p (k n) -> p k n", k=KC)[:, :, s0:s0 + sn],
                        in_=xT.ap()[:, t0 + s0:t0 + s0 + sn].rearrange(
                            "(k p) n -> p k n", p=128
                        ),
                    )
                    s0 += sn

                # ---- GEMM1 + GELU: hT[f, tokens] = gelu(W1.T @ x)
                # W1 streamed once per window in 256-wide panels.
                h_tiles = []
                for fo in range(F // 256):
                    w1_sb = w1p.tile([128, KC * 256], F32R, tag="w1")
                    # First panels of the first window ride the scalar-engine
                    # HWDGE ring, parallel to the sync ring carrying x, to
                    # shorten the kernel-start fill before the first matmul.
                    dma_eng = nc.scalar if (first_window and fo < 2) else nc.sync
                    dma_eng.dma_start(
                        out=w1_sb[:].rearrange("p (k f) -> p k f", k=KC),
                        in_=w1.ap()[:, fo * 256:(fo + 1) * 256].rearrange(
                            "(k p) f -> p k f", p=128
                        ),
                    )
                    for fi in range(2):
                        hT = hp.tile([128, wn], F32R, tag=f"h{fo * 2 + fi}")
                        s0 = 0
                        for sn in subs:
                            ph = ps1.tile([128, sn], F32, tag="ph")
                            for k in range(KC):
                                nc.tensor.matmul(
                                    ph[:],
                                    lhsT=w1_sb[:, k * 256 + fi * 128:
                                               k * 256 + (fi + 1) * 128],
                                    rhs=x_sb[:, k * wn + s0:k * wn + s0 + sn],
                                    start=(k == 0),
                                    stop=(k == KC - 1),
                                )
                            nc.scalar.activation(hT[:, s0:s0 + sn], ph[:], gelu)
                            s0 += sn
                        h_tiles.append(hT)

                # ---- GEMM2: y[tok, co*512:+512] = hT.T @ W2 half, W2
                # streamed once per window in k-groups of 4; PSUM rotates per
                # (kg, m) group and partials accumulate in SBUF via DVE.
                for co in range(2):
                    yacc = [
                        yp.tile([128, 512], F32, tag=f"yam{m}", name=f"yam{m}")
                        for m in range(nm)
                    ]
                    for kg in range(KF // 4):
                        w2g = w2p.tile([128, 4 * 512], F32R, tag="w2g")
                        nc.sync.dma_start(
                            out=w2g[:].rearrange("p (k c) -> p k c", k=4),
                            in_=w2.ap()[kg * 512:(kg + 1) * 512,
                                        co * 512:(co + 1) * 512].rearrange(
                                "(k p) c -> p k c", p=128
                            ),
                        )
                        for m in range(nm):
                            py = ps2.tile([128, 512], F32, tag="py")
                            for j in range(4):
                                k = kg * 4 + j
                                nc.tensor.matmul(
                                    py[:],
                                    lhsT=h_tiles[k][:, m * 128:(m + 1) * 128],
                                    rhs=w2g[:, j * 512:(j + 1) * 512],
                                    start=(j == 0),
                                    stop=(j == 3),
                                )
                            if kg == 0:
                                nc.vector.tensor_copy(yacc[m][:], py[:])
                            else:
                                nc.vector.tensor_add(yacc[m][:], yacc[m][:], py[:])
                    for m in range(nm):
                        nc.sync.dma_start(
                            out=y.ap()[t0 + m * 128:t0 + (m + 1) * 128,
                                       co * 512:(co + 1) * 512],
                            in_=yacc[m][:],
                        )
                t0 += wn
                first_window = False
    nc.compile()
    return nc


def kernel(x, Wg, W1, W2):
    global LAST_EXEC_TIME_NS, LAST_RESULTS
    x = np.asarray(x, dtype=np.float32)
    Wg = np.asarray(Wg, dtype=np.float32)
    W1 = np.asarray(W1, dtype=np.float32)
    W2 = np.asarray(W2, dtype=np.float32)
    B, T, _ = x.shape
    ntok = B * T
    xf = x.reshape(ntok, C)

    # ---- router (replicated gate, fp64 for stable selection)
    logits = xf.astype(np.float64) @ Wg.astype(np.float64)
    logits -= logits.max(-1, keepdims=True)
    probs = np.exp(logits)
    probs /= probs.sum(-1, keepdims=True)
    top2 = np.argsort(-probs, axis=-1, kind="stable")[:, :TOP_K]       # [ntok, 2]
    w12 = np.take_along_axis(probs, top2, axis=-1)
    w12 = w12 / w12.sum(-1, keepdims=True)                             # [ntok, 2]

    # aux load-balancing loss
    f_frac = np.bincount(top2.ravel(), minlength=E) / (ntok * TOP_K)
    P_mean = probs.mean(axis=0)
    aux_loss = np.float32(E * (f_frac * P_mean).sum())

    # ---- dispatch: gather each expert's tokens, pad to shared capacity
    token_lists = [np.nonzero((top2 == e).any(-1))[0] for e in range(E)]
    maxcnt = max(len(t) for t in token_lists)
    chunks = _chunks_for(maxcnt)
    cap = sum(chunks)

    in_maps = []
    for e in range(E):
        tl = token_lists[e]
        xe = np.zeros((C, cap), np.float32)
        xe[:, :len(tl)] = xf[tl].T
        in_maps.append({
            "xT": xe,
            "w1": np.ascontiguousarray(W1[e]),
            "w2": np.ascontiguousarray(W2[e]),
        })

    nc = _build(chunks)
    res = run_bass_kernel_spmd(nc, in_maps, list(range(N_CORES)))
    LAST_EXEC_TIME_NS = res.exec_time_ns
    LAST_RESULTS = res

    # ---- combine: out[t] = sum_k w12[t,k] * y_{expert k}[t]
    out = np.zeros((ntok, C), np.float64)
    for e in range(E):
        tl = token_lists[e]
        ye = res.results[e]["y"][:len(tl)].astype(np.float64)
        we = np.where(top2[tl, 0] == e, w12[tl, 0], w12[tl, 1])[:, None]
        out[tl] += we * ye
    return out.reshape(B, T, C).astype(np.float32), aux_loss
